# revision 1
# baseline (speedup 1.0000x reference)
"""Trainium2 Bass kernel for nn_CustomLSTM (B=256, T=1024, I=64, H=256, O=8).

Strategy: data-parallel over batch across 8 cores (32 batch rows each).
Per core, the LSTM recurrence runs with everything in feature-on-partition
("transposed") layout:
  - h^T, c^T as [128 partitions(H-dims), 2 k-tiles, B] tiles
  - gate pre-activations accumulate in a PSUM ring, one bank per timestep
  - x-projections (+ bias, via an appended ones-row on x) are computed in
    bulk S steps ahead with Wx stationary, so the serial path only streams
    the Wh tiles
  - C batch-chains run offset by one timestep in a ladder so ACT/VEC work
    of one chain hides under PE work of the others
Output projection h_T @ W_ho runs on-device; b_ho is added on the host.

This file is self-contained: shapes/sharding are hardcoded to the problem.
"""

import sys

sys.path.insert(0, "/opt/trn_rl_repo")

import numpy as np

import concourse.bass as bass
import concourse.mybir as mybir
from concourse.tile import TileContext
from concourse.vector_clock import ScopedClock, VectorClock

# ----------------------------------------------------------------------------
# Problem constants (full problem, then per-core)
# ----------------------------------------------------------------------------
B_FULL, T, I, H, O = 256, 1024, 64, 256, 8
NCORES = 8
B = B_FULL // NCORES          # 32 batch rows per core
G = 4 * H                     # 1024 gate pre-activations
KT = H // 128                 # 2 k-tiles for the h-part
MT = G // 128                 # 8 m-tiles of gate columns

# Tunables
C = 2  # batch chains (ladder depth)
Bc = B // C                   # batch per chain
# S=1: one step's x-projection per pass. Smooths PE occupancy (no 8x4
# matmul burst ever sits in front of the h-phase in PE program order);
# HW A/B over 5 paired rounds: ~2-4% faster than S=4, identical numerics.
S = 1                         # x-projection lookahead (steps per x-phase)
R = S + C                     # PSUM ring slots (each = 1 bank); R + C <= 8
SC = 64                       # x DMA superchunk (steps per DMA)
import os as _os
# KERNEL_DT: "bf16" (default; both paths bf16, W_ho/psum/c stay fp32),
#            "mixed" (x path fp32), "fp32" (both)
_mode = _os.environ.get("KERNEL_DT", "bf16")
HDT = mybir.dt.float32 if _mode == "fp32" else mybir.dt.bfloat16
XDT = mybir.dt.bfloat16 if _mode == "bf16" else mybir.dt.float32
SIG_SPLIT = False             # True: sig(f,i) + sig(o) separate; False: one sig op
M_OUTER = False               # True: loop m outer / k inner in the h-phase
# TANH_TRICK: sigmoid(g)=(tanh(g/2)+1)/2 with the 1/2's pre-folded into the
# weights: ONE tanh ACT op covers all 8 gate tiles. h-state is stored as
# 2h (Wh, W_ho pre-halved); c kept true via an off-critical-path halving.
TANH_TRICK = True

# m-tile permutation of gate columns: [f0 f1 i0 i1 o0 o1 c0 c1]
# (f,i,o sigmoid tiles contiguous first, then the two tanh tiles)
# reference gate column order is [f(0:256) i(256:512) c(512:768) o(768:1024)]
M_PERM = [0, 1, 2, 3, 6, 7, 4, 5]  # source m-tile index for each packed slot


# ----------------------------------------------------------------------------
# Tile walrus workaround: this container's walrus accepts at most ONE sync
# wait per instruction.  (a) patch the TileContext tail drain to spread its
# waits over per-proc SP nops; (b) after build, hoist excess waits from any
# instruction onto same-engine nops placed immediately before it.
# ----------------------------------------------------------------------------
def _patched_drain_and_barrier(self, tick_clock, wait_clock):
    nc = self.nc
    g = tick_clock.global_clock
    n = len(g)
    for p in range(n):
        if g[p] == 0:
            continue
        vc = VectorClock([g[q] if q == p else 0 for q in range(n)])
        nop = nc.sync.nop(nofuse=True)
        wait_clock.add_sem_waits(nop.ins, ScopedClock({None: vc}))
    nc.sync.drain()
    nc.all_engine_barrier()
    assert self.sems is not None
    popped = nc._tile_sem_poison_stack.pop()
    assert popped is self._sem_poison
    nc.clear_and_free_semaphores(list(self.sems.allocated().values()))
    nc.all_engine_barrier()


def apply_tile_patch():
    TileContext._drain_and_barrier = _patched_drain_and_barrier


def legalize_waits(nc, limit=1):
    """Hoist excess sem waits (>limit per instruction) onto same-engine nops
    inserted immediately before the instruction."""
    eng_builders = {
        mybir.EngineType.PE: nc.tensor,
        mybir.EngineType.DVE: nc.vector,
        mybir.EngineType.Activation: nc.scalar,
        mybir.EngineType.Pool: nc.gpsimd,
        mybir.EngineType.SP: nc.sync,
    }
    n_hoisted = 0
    for f in nc.m.functions:
        for bb in f.blocks:
            snapshot = list(bb.instructions)
            fixes = []  # (index, inst, excess_waits)
            for idx, inst in enumerate(snapshot):
                si = inst.sync_info
                waits = list(si.on_wait) if si and si.on_wait else []
                if len(waits) > limit:
                    fixes.append((idx, inst, waits))
            if not fixes:
                continue
            # create nops via the engine builders (they append to cur_bb;
            # pop them back off to place manually)
            out = []
            prev = 0
            for idx, inst, waits in fixes:
                out.extend(snapshot[prev:idx])
                keep = waits[-limit:]
                excess = waits[:-limit]
                for w in excess:
                    builder = eng_builders[inst.engine]
                    nop_bi = builder.nop(nofuse=True)
                    nop_inst = nop_bi.ins
                    # remove from wherever the builder appended it
                    cur = nc.cur_bb.bb
                    assert cur.instructions[-1] is nop_inst
                    cur.instructions.pop()
                    nop_inst.sync_info = mybir.SyncInfo(on_wait=[w], on_update=[])
                    out.append(nop_inst)
                    n_hoisted += 1
                inst.sync_info = mybir.SyncInfo(
                    on_wait=keep, on_update=list(inst.sync_info.on_update or [])
                )
                out.append(inst)
                prev = idx + 1
            out.extend(snapshot[prev:])
            bb.instructions = out
    return n_hoisted


# ----------------------------------------------------------------------------
# Kernel build
# ----------------------------------------------------------------------------
def build_nc(t_steps=T, hdt=None, xdt=None, dt=None):
    """Build the per-core Bass program. Returns nc."""
    if dt is not None:
        hdt = xdt = dt
    hdt = HDT if hdt is None else hdt
    xdt = XDT if xdt is None else xdt
    apply_tile_patch()
    fp32 = mybir.dt.float32
    Af = mybir.ActivationFunctionType

    nc = bass.Bass()
    xT_d = nc.dram_tensor("xT", [I + 1, t_steps * B], xdt, kind="ExternalInput")
    Wh_d = nc.dram_tensor("Wh", [128, KT * G], hdt, kind="ExternalInput")
    Wx_d = nc.dram_tensor("Wx", [I + 1, G], xdt, kind="ExternalInput")
    Who_d = nc.dram_tensor("Who", [128, KT * O], fp32, kind="ExternalInput")
    y_d = nc.dram_tensor("y", [B, O], fp32, kind="ExternalOutput")

    n_pass = t_steps + C - 1

    with TileContext(nc) as tc:
        with (
            tc.tile_pool(name="wpool", bufs=1) as wpool,
            tc.tile_pool(name="state", bufs=1) as state,
            tc.tile_pool(name="xbuf", bufs=3) as xbuf,
            tc.tile_pool(name="gbuf", bufs=2 * C + 1) as gbuf,
            tc.tile_pool(name="tbuf", bufs=2 * C + 2) as tbuf,
            tc.tile_pool(name="ring", bufs=1, space="PSUM") as ringp,
            tc.tile_pool(name="ypsum", bufs=1, space="PSUM") as ypool,
            tc.tile_pool(name="ysb", bufs=1) as ysbp,
        ):
            # --- weights ---
            Wh_s = wpool.tile([128, KT * G], hdt, tag="Wh_s")
            nc.sync.dma_start(Wh_s[:], Wh_d[:])
            Wx_s = wpool.tile([I + 1, G], xdt, tag="Wx_s")
            nc.sync.dma_start(Wx_s[:], Wx_d[:])
            Who_s = wpool.tile([128, KT * O], fp32, tag="Who_s")
            nc.sync.dma_start(Who_s[:], Who_d[:])

            # --- state (per chain) ---
            h_t = []
            c_t = []
            for j in range(C):
                hj = state.tile([128, KT, Bc], hdt, tag=f"h{j}")
                cj = state.tile([128, KT, Bc], fp32, tag=f"c{j}")
                nc.vector.memset(hj[:], 0.0)
                nc.vector.memset(cj[:], 0.0)
                h_t.append(hj)
                c_t.append(cj)

            # --- psum ring: one bank per in-flight timestep ---
            ring = [
                ringp.tile([128, MT, B], fp32, tag=f"ring{r}", name=f"ring{r}")
                for r in range(R)
            ]

            # --- x superchunk tiles, DMA'd ahead ---
            n_chunk = (t_steps + SC - 1) // SC
            xch = {}

            def fetch_chunk(ci):
                if ci in xch or ci >= n_chunk:
                    return
                cols = min(SC, t_steps - ci * SC) * B
                xt = xbuf.tile([I + 1, SC * B], xdt, tag="xch")
                nc.sync.dma_start(
                    xt[:, 0:cols], xT_d[:, ci * SC * B : ci * SC * B + cols]
                )
                xch[ci] = xt

            fetch_chunk(0)
            fetch_chunk(1)

            # --- main ladder ---
            for p in range(n_pass):
                # prefetch next x superchunk
                if p % SC == 0 and p < t_steps:
                    fetch_chunk(p // SC + 1)

                # x-phase: bulk x-projection (+bias) for steps [p, p+S)
                if p % S == 0 and p < t_steps:
                    for t in range(p, min(p + S, t_steps)):
                        slot = ring[t % R]
                        xt = xch[t // SC]
                        rhs = xt[:, (t % SC) * B : (t % SC) * B + B]
                        for m in range(MT):
                            nc.tensor.matmul(
                                slot[:, m, :],
                                Wx_s[:, m * 128 : (m + 1) * 128],
                                rhs,
                                start=(m == 0),
                                stop=False,
                                skip_group_check=True,
                            )

                # serial h-phase: all active chains, shared weight tiles
                active = [j for j in range(C) if 0 <= p - j < t_steps]
                if M_OUTER:
                    km_order = [(k, m) for m in range(MT) for k in range(KT)]
                else:
                    km_order = [(k, m) for k in range(KT) for m in range(MT)]
                last_km = km_order[-1]
                for k, m in km_order:
                    lhsT = Wh_s[:, k * G + m * 128 : k * G + (m + 1) * 128]
                    for j in active:
                        slot = ring[(p - j) % R]
                        nc.tensor.matmul(
                            slot[:, m, j * Bc : (j + 1) * Bc],
                            lhsT,
                            h_t[j][:, k, :],
                            start=False,
                            stop=(j == C - 1 and (k, m) == last_km),
                            skip_group_check=True,
                        )

                # nonlinear phase per active chain.
                # split ACT ops so VEC work can start as early as possible:
                #   sig(f,i) -> t1=f*c (VEC) || tanh(chat) -> t2=i*chat,
                #   sig(o) overlaps the adds; tanh(c) then h=o*th.
                for j in active:
                    slot = ring[(p - j) % R]
                    if TANH_TRICK:
                        # one tanh over all 8 gate tiles; tf/ti/to are
                        # tanh(g/2) = 2*sigmoid(g)-1, chat is true tanh.
                        g8 = gbuf.tile([128, 8, Bc], fp32, tag="g8")
                        nc.scalar.activation(
                            g8[:], slot[:, :, j * Bc : (j + 1) * Bc], Af.Tanh
                        )
                        u1 = tbuf.tile([128, KT, Bc], fp32, tag="u1")
                        u2 = tbuf.tile([128, KT, Bc], fp32, tag="u2")
                        v2 = tbuf.tile([128, KT, Bc], fp32, tag="v2")
                        th = tbuf.tile([128, KT, Bc], fp32, tag="th")
                        mlt = mybir.AluOpType.mult
                        addo = mybir.AluOpType.add
                        # u1 = (tf+1)*c = 2*f*c ; u2 = (ti+1)*chat = 2*i*chat
                        nc.vector.scalar_tensor_tensor(
                            u1[:], g8[:, 0:2, :], 1.0, c_t[j][:], addo, mlt
                        )
                        nc.vector.scalar_tensor_tensor(
                            u2[:], g8[:, 2:4, :], 1.0, g8[:, 6:8, :], addo, mlt
                        )
                        nc.vector.tensor_add(v2[:], u1[:], u2[:])  # = 2*c_new
                        # th = tanh(c_new) via free input scale
                        nc.scalar.activation(th[:], v2[:], Af.Tanh, scale=0.5)
                        # h2 = (to+1)*th = 2*o*th = 2*h  (Wh, W_ho pre-halved)
                        nc.vector.scalar_tensor_tensor(
                            h_t[j][:], g8[:, 4:6, :], 1.0, th[:], addo, mlt
                        )
                        # true c for the next step (off critical path)
                        nc.vector.tensor_scalar_mul(c_t[j][:], v2[:], 0.5)
                        continue
                    gch = gbuf.tile([128, 2, Bc], fp32, tag="gch")
                    t1 = tbuf.tile([128, KT, Bc], fp32, tag="t1")
                    t2 = tbuf.tile([128, KT, Bc], fp32, tag="t2")
                    th = tbuf.tile([128, KT, Bc], fp32, tag="th")
                    if SIG_SPLIT:
                        gfi = gbuf.tile([128, 4, Bc], fp32, tag="gfi")
                        go = gbuf.tile([128, 2, Bc], fp32, tag="go")
                        nc.scalar.activation(
                            gfi[:], slot[:, 0:4, j * Bc : (j + 1) * Bc], Af.Sigmoid
                        )
                        nc.scalar.activation(
                            gch[:], slot[:, 6:8, j * Bc : (j + 1) * Bc], Af.Tanh
                        )
                        nc.vector.tensor_mul(t1[:], gfi[:, 0:2, :], c_t[j][:])
                        nc.vector.tensor_mul(t2[:], gfi[:, 2:4, :], gch[:])
                        nc.scalar.activation(
                            go[:], slot[:, 4:6, j * Bc : (j + 1) * Bc], Af.Sigmoid
                        )
                        f_o = go
                    else:
                        gsig = gbuf.tile([128, 6, Bc], fp32, tag="gsig")
                        nc.scalar.activation(
                            gsig[:], slot[:, 0:6, j * Bc : (j + 1) * Bc], Af.Sigmoid
                        )
                        nc.scalar.activation(
                            gch[:], slot[:, 6:8, j * Bc : (j + 1) * Bc], Af.Tanh
                        )
                        nc.vector.tensor_mul(t1[:], gsig[:, 0:2, :], c_t[j][:])
                        nc.vector.tensor_mul(t2[:], gsig[:, 2:4, :], gch[:])
                        f_o = gsig[:, 4:6, :]
                    nc.vector.tensor_add(c_t[j][:], t1[:], t2[:])
                    # h = o * tanh(c)
                    nc.scalar.activation(th[:], c_t[j][:], Af.Tanh)
                    nc.vector.tensor_mul(h_t[j][:], f_o[:] if SIG_SPLIT else f_o, th[:])

            # --- output projection: y = h_T @ W_ho (bias on host) ---
            # cast h to fp32 so the final projection is full precision
            # (W_ho stays fp32); reuse ring slot j's bank as the y psum.
            for j in range(C):
                hc = ysbp.tile([128, KT, Bc], fp32, tag=f"hc{j}", name=f"hc{j}")
                nc.vector.tensor_copy(hc[:], h_t[j][:])
                yp = ring[j][0:Bc, 0, 0:O]
                for k in range(KT):
                    nc.tensor.matmul(
                        yp[:],
                        hc[:, k, :],
                        Who_s[:, k * O : (k + 1) * O],
                        start=(k == 0),
                        stop=(k == KT - 1),
                        skip_group_check=True,
                    )
                ys = ysbp.tile([Bc, O], fp32, tag=f"ys{j}")
                nc.vector.tensor_copy(ys[:], yp[:])
                nc.sync.dma_start(y_d[j * Bc : (j + 1) * Bc, :], ys[:])

    n = legalize_waits(nc, limit=1)
    return nc


# ----------------------------------------------------------------------------
# Host-side packing
# ----------------------------------------------------------------------------
def _np_dt(dt):
    import ml_dtypes

    return np.float32 if dt == mybir.dt.float32 else ml_dtypes.bfloat16


def pack_weights(W_f, b_f, W_i, b_i, W_c, b_c, W_o, b_o, W_ho, hdt=None, xdt=None, t_steps=T):
    """Build Wh [128, KT*G], Wx [I+1, G], Who [128, KT*O] in packed layout."""
    np_h = _np_dt(HDT if hdt is None else hdt)
    np_x = _np_dt(XDT if xdt is None else xdt)
    Wg = np.concatenate([W_f, W_i, W_c, W_o], axis=1).astype(np.float32)  # [I+H, 4H]
    bg = np.concatenate([b_f, b_i, b_c, b_o], axis=0).astype(np.float32)  # [4H]
    # column m-tile permutation
    cols = np.concatenate(
        [np.arange(m * 128, (m + 1) * 128) for m in M_PERM]
    )
    Wg_p = Wg[:, cols]
    bg_p = bg[cols]
    # h-part rows 0:H (combined = [h, x]); x-part rows H:H+I
    Wh = Wg_p[0:H, :]                       # [256, 1024]
    Wx = Wg_p[H : H + I, :]                 # [64, 1024]
    Who = W_ho.astype(np.float32)           # [256, 8]
    if TANH_TRICK:
        # sigmoid(g) = (tanh(g/2)+1)/2: halve f,i,o gate columns (slots
        # 0:6 of the m-tile permutation) incl. bias; h is stored as 2h so
        # all Wh rows and W_ho are halved as well.
        colscale = np.ones((G,), np.float32)
        colscale[0 : 6 * 128] = 0.5
        Wh = Wh * colscale[None, :] * 0.5
        Wx = Wx * colscale[None, :]
        bg_p = bg_p * colscale
        Who = Who * 0.5
    Wx_aug = np.concatenate([Wx, bg_p[None, :]], axis=0)  # [65, 1024]
    # k-tiles side by side: [128, KT*G]
    Wh_pk = np.concatenate([Wh[k * 128 : (k + 1) * 128, :] for k in range(KT)], axis=1)
    Who_pk = np.concatenate(
        [Who[k * 128 : (k + 1) * 128, :] for k in range(KT)], axis=1
    )  # [128, 16]
    return Wh_pk.astype(np_h), Wx_aug.astype(np_x), Who_pk.astype(np.float32)


def pack_x(x, xdt=None, t_steps=T):
    """x [B_FULL, T, I] -> list of per-core xT [I+1, T*B] (with ones row)."""
    npdt = _np_dt(XDT if xdt is None else xdt)
    outs = []
    for c in range(NCORES):
        xs = np.asarray(x[c * B : (c + 1) * B, :t_steps, :], dtype=np.float32)
        xt = np.ascontiguousarray(xs.transpose(2, 1, 0))  # [I, T, B]
        ones = np.ones((1, t_steps, B), np.float32)
        xa = np.concatenate([xt, ones], axis=0).reshape(I + 1, t_steps * B)
        outs.append(xa.astype(npdt))
    return outs


# ----------------------------------------------------------------------------
# Public entry point
# ----------------------------------------------------------------------------
_CACHE = {}


def _get_nc(t_steps=T):
    key = (t_steps, str(HDT), str(XDT))
    if key not in _CACHE:
        _CACHE[key] = build_nc(t_steps)
    return _CACHE[key]


def kernel(x, W_f, b_f, W_i, b_i, W_c, b_c, W_o, b_o, W_ho, b_ho):
    from concourse.bass_utils import run_bass_kernel_spmd

    x = np.asarray(x)
    nc = _get_nc()
    Wh_pk, Wx_aug, Who_pk = pack_weights(
        W_f, b_f, W_i, b_i, W_c, b_c, W_o, b_o, W_ho
    )
    xs = pack_x(x)
    in_maps = [
        {"xT": xs[c], "Wh": Wh_pk, "Wx": Wx_aug, "Who": Who_pk}
        for c in range(NCORES)
    ]
    res = run_bass_kernel_spmd(nc, in_maps, list(range(NCORES)))
    y = np.concatenate([res.results[c]["y"] for c in range(NCORES)], axis=0)
    return (y + np.asarray(b_ho, np.float32)[None, :]).astype(np.float32)

